# revision 1
# baseline (speedup 1.0000x reference)
"""Trainium2 Bass kernel for LongcatFlash MoE experts (expert-parallel, 8 cores).

Problem: T=4096 tokens, H=1024, I=512, 32 routed + 8 zero (identity) experts,
top-4 routing, per-expert capacity 768.

Strategy (sharding_hint = expert parallelism):
  - Host: compute routing (stable sort by expert, capacity clip), permute
    tokens to their expert's core (the "all-to-all"), build per-core packed
    activation buffers with tokens on the GEMM free dimension.
  - Device (8 cores, SPMD): each core owns 4 routed experts; per expert run
    the gated MLP as tiled matmuls:
        gu[o, c]  = sum_h guT[h, o] * xT[h, c]      (o = 2I rows, c = tokens)
        mid[i, c] = silu(gate[i, c]) * up[i, c]
        y[h, c]   = sum_i dnT[i, h] * mid[i, c]
    Tokens live on the free dim (N <= 512 per matmul), weights are the
    stationary operand.
  - Host: gather per-assignment outputs, scale by router weight, scatter-add
    back per token, add the zero-expert weighted-identity term.
"""

import math
import os

import numpy as np

N_CORES = 8
R = 32  # routed experts
E_PER_CORE = R // N_CORES  # 4
CAPACITY = 768
H = 1024
I_DIM = 512
HT = H // 128  # 8 h-tiles
OT = 2 * I_DIM // 128  # 8 o-tiles of gate_up
IT = I_DIM // 128  # 4 i-tiles

# precision mode: "bf16" (fast, rel err ~4e-4) or "f32r" (fp32 storage,
# FP22 matmul, rel err ~3e-5 but ~1.4x slower: 2 PE cycles/row + 2x DMA)
PREC = os.environ.get("MOE_PREC", "bf16")

LAST_RUN = {}  # filled with exec_time_ns etc. for test harness use


def _route(idx, wts, n_tok):
    """Replicates the reference's capacity-buffer routing exactly.

    Returns per-assignment (expert, token, weight, slot, flat_index) for kept
    routed assignments, sorted by expert (stable), plus zero-expert weights.
    """
    K = idx.shape[1]
    A = n_tok * K
    flat_e = idx.reshape(-1).astype(np.int64)
    flat_t = np.repeat(np.arange(n_tok, dtype=np.int64), K)
    flat_w = wts.reshape(-1)
    order = np.argsort(flat_e, kind="stable")
    se = flat_e[order]
    st = flat_t[order]
    sw = flat_w[order]
    counts = np.bincount(flat_e, minlength=R + 8)
    starts = np.cumsum(counts) - counts
    pos = np.arange(A, dtype=np.int64) - starts[se]
    valid = (se < R) & (pos < CAPACITY)
    zero_w = np.where(idx >= R, wts, 0.0).sum(axis=1)
    return (
        se[valid],
        st[valid],
        sw[valid],
        pos[valid],
        order[valid],
        zero_w,
    )


def _chunks(S):
    n = (S + 511) // 512
    base = S // n
    rem = S - base * n
    out = []
    c0 = 0
    for i in range(n):
        cn = base + (1 if i < rem else 0)
        out.append((c0, cn))
        c0 += cn
    return out


_BUILD_CACHE = {}


def _build_bass(S, prec):
    import concourse.bacc as bacc
    import concourse.bass as bass
    import concourse.mybir as mybir
    from concourse import tile

    key = (S, prec)
    if key in _BUILD_CACHE:
        return _BUILD_CACHE[key]

    FT = mybir.dt.float32
    if prec == "bf16":
        dram_dt = mybir.dt.bfloat16
        sb_dt = mybir.dt.bfloat16
        mid_dt = mybir.dt.bfloat16
        out_dt = mybir.dt.bfloat16
    else:
        dram_dt = mybir.dt.float32r
        sb_dt = mybir.dt.float32r
        mid_dt = mybir.dt.float32r
        out_dt = mybir.dt.float32

    chunks = _chunks(S)

    nc = bacc.Bacc(None)
    xt_d = nc.declare_dram_parameter("xt", [E_PER_CORE, HT, 128, S], dram_dt, isOutput=False)
    gu_d = nc.declare_dram_parameter("guw", [E_PER_CORE, HT, 128, 1024], dram_dt, isOutput=False)
    dn_d = nc.declare_dram_parameter("dnw", [E_PER_CORE, IT, 128, 1024], dram_dt, isOutput=False)
    yt_d = nc.declare_dram_parameter("yt", [E_PER_CORE, 128, HT * S], out_dt, isOutput=True)

    silu_fn = mybir.ActivationFunctionType.Silu

    # bf16 tiles are half-size; the f32r fallback needs smaller pools to fit
    # SBUF (~192 KB/partition usable)
    gu_bufs = 2 * HT if prec == "bf16" else HT + IT
    y_bufs = 4 if prec == "bf16" else 2
    with tile.TileContext(nc) as tc:
        with (
            tc.tile_pool(name="xpool", bufs=2 * HT) as xpool,
            tc.tile_pool(name="gupool", bufs=gu_bufs) as gupool,
            tc.tile_pool(name="dnpool", bufs=2 * IT) as dnpool,
            tc.tile_pool(name="midpool", bufs=2 * IT * len(chunks)) as midpool,
            # sil tiles are ACT-written; unique slots (no reuse) keep the
            # Activation instruction at a single sync-wait (AC struct limit 1)
            tc.tile_pool(name="silpool", bufs=E_PER_CORE * IT * len(chunks)) as silpool,
            tc.tile_pool(name="ypool", bufs=y_bufs) as ypool,
            tc.tile_pool(name="pgpool", bufs=3, space="PSUM") as pgpool,
            tc.tile_pool(name="pupool", bufs=3, space="PSUM") as pupool,
            tc.tile_pool(name="pypool", bufs=2, space="PSUM") as pypool,
        ):
            for e in range(E_PER_CORE):
                # interleave x / gate_up stripe loads so the first matmul can
                # start as soon as stripe 0 lands; split issue across engines
                # (DMA trigger is ~0.6us each on one sequencer)
                xts = []
                guts = []
                for h in range(HT):
                    tx = xpool.tile([128, S], sb_dt, tag="xt")
                    nc.scalar.dma_start(tx[:], xt_d[e, h])
                    xts.append(tx)
                    tg = gupool.tile([128, 1024], sb_dt, tag="gu")
                    nc.sync.dma_start(tg[:], gu_d[e, h])
                    guts.append(tg)
                dnts = []
                for i in range(IT):
                    t = dnpool.tile([128, 1024], sb_dt, tag="dn")
                    nc.sync.dma_start(t[:], dn_d[e, i])
                    dnts.append(t)

                mids = {}
                ywide = ypool.tile([128, HT * S], out_dt, tag="yo")
                for ci, (c0, cn) in enumerate(chunks):
                    for oi in range(IT):
                        pg = pgpool.tile([128, cn], FT, tag="pg")
                        pu = pupool.tile([128, cn], FT, tag="pu")
                        for h in range(HT):
                            nc.tensor.matmul(
                                pg[:],
                                guts[h][:, oi * 128 : (oi + 1) * 128],
                                xts[h][:, c0 : c0 + cn],
                                start=(h == 0),
                                stop=(h == HT - 1),
                            )
                        for h in range(HT):
                            nc.tensor.matmul(
                                pu[:],
                                guts[h][:, (IT + oi) * 128 : (IT + oi + 1) * 128],
                                xts[h][:, c0 : c0 + cn],
                                start=(h == 0),
                                stop=(h == HT - 1),
                            )
                        sil = silpool.tile([128, cn], FT, tag="sil")
                        nc.scalar.activation(sil[:], pg[:], silu_fn)
                        m = midpool.tile([128, cn], mid_dt, tag="mid")
                        nc.vector.scalar_tensor_tensor(
                            m[:], pu[:], 1.0, sil[:],
                            mybir.AluOpType.mult, mybir.AluOpType.mult,
                        )
                        mids[(ci, oi)] = m
                    for h in range(HT):
                        py = pypool.tile([128, cn], FT, tag="py")
                        for i in range(IT):
                            nc.tensor.matmul(
                                py[:],
                                dnts[i][:, h * 128 : (h + 1) * 128],
                                mids[(ci, i)][:],
                                start=(i == 0),
                                stop=(i == IT - 1),
                            )
                        nc.vector.tensor_copy(
                            ywide[:, h * S + c0 : h * S + c0 + cn], py[:]
                        )
                        if ci == len(chunks) - 1 and h % 2 == 1:
                            h0 = h - 1
                            nc.gpsimd.dma_start(
                                yt_d[e, :, h0 * S : (h + 1) * S],
                                ywide[:, h0 * S : (h + 1) * S],
                            )

    nc.finalize()
    _BUILD_CACHE[key] = nc
    return nc


def _install_trace_shims():
    """Make trace=True usable in this image: provide the NTFF hook module and
    neutralize the artifact upload (no bucket access needed for local use)."""
    import sys
    import types

    try:
        import antenv.axon_hooks  # noqa: F401
    except ImportError:
        hook = None
        try:
            from trn_agent_boot.trn_boot import _ntff_profile_via_ctypes

            hook = _ntff_profile_via_ctypes("/opt/axon/libaxon_pjrt.so")
        except Exception:
            hook = None
        mod = types.ModuleType("antenv.axon_hooks")
        mod._hook = hook
        mod.get_axon_ntff_profile_hook = lambda: mod._hook
        mod.set_axon_ntff_profile_hook = lambda h: setattr(mod, "_hook", h)
        sys.modules["antenv.axon_hooks"] = mod

    import concourse.bass_utils as bu

    orig_upload = bu.upload_artifacts

    def safe_upload(tmpdir):
        try:
            return orig_upload(tmpdir)
        except Exception:
            return tmpdir

    bu.upload_artifacts = safe_upload


def kernel(**inputs):
    from concourse.bass_utils import run_bass_kernel_spmd

    hidden = np.ascontiguousarray(np.asarray(inputs["hidden_states"], dtype=np.float32))
    idx = np.asarray(inputs["top_k_index"]).astype(np.int64)
    wts = np.asarray(inputs["top_k_weights"], dtype=np.float32)
    gup = np.asarray(inputs["gate_up_proj"], dtype=np.float32)
    dnp = np.asarray(inputs["down_proj"], dtype=np.float32)

    n_tok = hidden.shape[0]
    K = idx.shape[1]

    ve, vt, vw, vp, va, zero_w = _route(idx, wts, n_tok)
    cnts = np.bincount(ve, minlength=R)
    maxc = int(cnts.max())
    # N multiple of 64 elements keeps the PE moving-operand stream at full
    # rate (440 measured 231 ns/MM vs 448 at 202 ns/MM)
    S = max(256, ((maxc + 63) // 64) * 64)

    if PREC == "bf16":
        import ml_dtypes

        io_np = ml_dtypes.bfloat16
    else:
        io_np = np.float32

    # per-expert slices in the expert-sorted assignment arrays
    estarts = np.cumsum(cnts) - cnts

    in_maps = []
    for c in range(N_CORES):
        xt = np.zeros((E_PER_CORE, HT, 128, S), dtype=io_np)
        for le in range(E_PER_CORE):
            ge = c * E_PER_CORE + le
            s0, cnt = estarts[ge], cnts[ge]
            if cnt == 0:
                continue
            toks = vt[s0 : s0 + cnt]
            # [cnt, H] -> [H, cnt] -> tiles [HT, 128, cnt]
            xbuf = hidden[toks].T.reshape(HT, 128, cnt)
            xt[le, :, :, :cnt] = xbuf.astype(io_np)
        guw = (
            gup[c * E_PER_CORE : (c + 1) * E_PER_CORE]
            .transpose(0, 2, 1)  # [4, H, 2I]
            .reshape(E_PER_CORE, HT, 128, 1024)
            .astype(io_np)
        )
        dnw = (
            dnp[c * E_PER_CORE : (c + 1) * E_PER_CORE]
            .transpose(0, 2, 1)  # [4, I, H]
            .reshape(E_PER_CORE, IT, 128, 1024)
            .astype(io_np)
        )
        in_maps.append({"xt": np.ascontiguousarray(xt),
                        "guw": np.ascontiguousarray(guw),
                        "dnw": np.ascontiguousarray(dnw)})

    nc = _build_bass(S, PREC)

    trace = bool(int(os.environ.get("KERNEL_TRACE", "0")))
    if trace:
        _install_trace_shims()
    res = run_bass_kernel_spmd(nc, in_maps, list(range(N_CORES)), trace=trace)
    LAST_RUN["exec_time_ns"] = res.exec_time_ns
    LAST_RUN["mean_exec_time_ns"] = res.mean_exec_time_ns
    LAST_RUN["instructions_and_trace"] = res.instructions_and_trace
    LAST_RUN["profile_json"] = res.profile_json

    # ---- combine on host ----
    out = hidden * zero_w[:, None].astype(np.float32)
    acc = np.zeros((n_tok * K, H), dtype=np.float32)
    for c in range(N_CORES):
        yt = np.asarray(res.results[c]["yt"]).astype(np.float32)  # [4, 128, HT*S]
        for le in range(E_PER_CORE):
            ge = c * E_PER_CORE + le
            s0, cnt = estarts[ge], cnts[ge]
            if cnt == 0:
                continue
            # [128, HT, S] -> [HT, 128, S] -> [H, S]
            y = yt[le].reshape(128, HT, S).transpose(1, 0, 2).reshape(H, S)[:, :cnt].T
            acc[va[s0 : s0 + cnt]] = y * vw[s0 : s0 + cnt, None]
    out += acc.reshape(n_tok, K, H).sum(axis=1)
    return out



# revision 2
# speedup vs baseline: 1.5069x; 1.5069x over previous
"""Trainium2 Bass kernel for LongcatFlash MoE experts (expert-parallel, 8 cores).

Problem: T=4096 tokens, H=1024, I=512, 32 routed + 8 zero (identity) experts,
top-4 routing, per-expert capacity 768.

Strategy (sharding_hint = expert parallelism):
  - Host: compute routing (stable sort by expert, capacity clip), permute
    tokens to their expert's core (the "all-to-all"), build per-core packed
    activation buffers with tokens on the GEMM free dimension.
  - Device (8 cores, SPMD): each core owns 4 routed experts; per expert run
    the gated MLP as tiled matmuls with tokens on the free dim:
        gu[o, c]  = sum_h guT[h, o] * xT[h, c]      (o = 2I rows, c = tokens)
        mid[i, c] = silu(gate[i, c]) * up[i, c]
        y[h, c]   = sum_i dnT[i, h] * mid[i, c]
  - Host: gather per-assignment outputs, scale by router weight, scatter-add
    back per token, add the zero-expert weighted-identity term.

Precision modes:
  - "fp8" (default): e4m3 weights/activations, DoubleRow matmuls (K=256 per
    MM, 2 fp8 MACs per PE cell per cycle).  Weights are pre-scaled by 128 on
    the host to sit in e4m3's precision sweet spot; the 1/128 is folded into
    the on-device silu/up scaling and the host-side combine.  TRN's e4m3 has
    max +-240 (= ml_dtypes.float8_e4m3, not the OCP "fn" variant).
  - "bf16": fallback, plain K=128 matmuls.
"""

import math
import os

import numpy as np

N_CORES = 8
R = 32  # routed experts
E_PER_CORE = R // N_CORES  # 4
CAPACITY = 768
H = 1024
I_DIM = 512
HT = H // 128  # 8 h-tiles
OT = 2 * I_DIM // 128  # 8 o-tiles of gate_up
IT = I_DIM // 128  # 4 i-tiles

WSCALE = 128.0  # fp8 weight pre-scale (power of 2: exact to undo)

PREC = os.environ.get("MOE_PREC", "fp8")

LAST_RUN = {}  # filled with exec_time_ns etc. for test harness use


def _route(idx, wts, n_tok):
    """Replicates the reference's capacity-buffer routing exactly.

    Returns per-assignment (expert, token, weight, slot, flat_index) for kept
    routed assignments, sorted by expert (stable), plus zero-expert weights.
    """
    K = idx.shape[1]
    A = n_tok * K
    flat_e = idx.reshape(-1).astype(np.int64)
    flat_t = np.repeat(np.arange(n_tok, dtype=np.int64), K)
    flat_w = wts.reshape(-1)
    order = np.argsort(flat_e, kind="stable")
    se = flat_e[order]
    st = flat_t[order]
    sw = flat_w[order]
    counts = np.bincount(flat_e, minlength=R + 8)
    starts = np.cumsum(counts) - counts
    pos = np.arange(A, dtype=np.int64) - starts[se]
    valid = (se < R) & (pos < CAPACITY)
    zero_w = np.where(idx >= R, wts, 0.0).sum(axis=1)
    return (
        se[valid],
        st[valid],
        sw[valid],
        pos[valid],
        order[valid],
        zero_w,
    )


def _chunks(S):
    n = (S + 511) // 512
    base = S // n
    rem = S - base * n
    out = []
    c0 = 0
    for i in range(n):
        cn = base + (1 if i < rem else 0)
        out.append((c0, cn))
        c0 += cn
    return out


_BUILD_CACHE = {}


def _build_fp8(S):
    """fp8 e4m3 DoubleRow pipeline: per expert
      4 gate psums + 4 up psums (4 DoubleRow MMs each, K=256),
      silu+mult -> fp8 mid, 8 down psums (2 DoubleRow MMs each),
      copy -> bf16 y, DMA out per h-pair.
    """
    import concourse.bacc as bacc
    import concourse.bass as bass
    import concourse.mybir as mybir
    from concourse import tile

    key = (S, "fp8")
    if key in _BUILD_CACHE:
        return _BUILD_CACHE[key]

    FT = mybir.dt.float32
    F8 = mybir.dt.float8e4
    BF = mybir.dt.bfloat16
    DR = mybir.MatmulPerfMode.DoubleRow
    silu_fn = mybir.ActivationFunctionType.Silu

    chunks = _chunks(S)
    assert len(chunks) == 1, "fp8 path assumes S <= 512"

    nc = bacc.Bacc(None)
    xt_d = nc.declare_dram_parameter("xt", [E_PER_CORE, 128, HT, S], F8, isOutput=False)
    gu_d = nc.declare_dram_parameter("guw", [E_PER_CORE, 128, HT, 1024], F8, isOutput=False)
    dn_d = nc.declare_dram_parameter("dnw", [E_PER_CORE, 128, IT, 1024], F8, isOutput=False)
    yt_d = nc.declare_dram_parameter("yt", [E_PER_CORE, 128, HT, S], BF, isOutput=True)

    inv = 1.0 / WSCALE

    with tile.TileContext(nc) as tc:
        with (
            tc.tile_pool(name="xpool", bufs=2) as xpool,
            tc.tile_pool(name="gupool", bufs=2) as gupool,
            tc.tile_pool(name="dnpool", bufs=2) as dnpool,
            tc.tile_pool(name="midpool", bufs=2) as midpool,
            # sil tiles are ACT-written; unique slots (no reuse) keep the
            # Activation instruction at a single sync-wait (AC struct limit 1)
            tc.tile_pool(name="silpool", bufs=E_PER_CORE * IT) as silpool,
            tc.tile_pool(name="ypool", bufs=2) as ypool,
            tc.tile_pool(name="pgpool", bufs=3, space="PSUM") as pgpool,
            tc.tile_pool(name="pupool", bufs=3, space="PSUM") as pupool,
            tc.tile_pool(name="pypool", bufs=2, space="PSUM") as pypool,
        ):
            for e in range(E_PER_CORE):
                xe = xpool.tile([128, HT, S], F8, tag="xt")
                ge = gupool.tile([128, HT, 1024], F8, tag="gu")
                de = dnpool.tile([128, IT, 1024], F8, tag="dn")
                # h-pair granular loads: first MM unblocks after pair 0 lands.
                # gu/dn pairs ride the sync (SP HWDGE) ring, x pairs the
                # scalar (ACT HWDGE) ring so the two streams overlap.
                for hh in range(HT // 2):
                    nc.sync.dma_start(
                        ge[:, 2 * hh : 2 * hh + 2, :], gu_d[e, :, 2 * hh : 2 * hh + 2, :]
                    )
                    nc.scalar.dma_start(
                        xe[:, 2 * hh : 2 * hh + 2, :], xt_d[e, :, 2 * hh : 2 * hh + 2, :]
                    )
                for ii in range(IT // 2):
                    nc.sync.dma_start(
                        de[:, 2 * ii : 2 * ii + 2, :], dn_d[e, :, 2 * ii : 2 * ii + 2, :]
                    )

                mid = midpool.tile([128, IT, S], F8, tag="mid")
                for oi in range(IT):
                    pg = pgpool.tile([128, S], FT, tag="pg")
                    pu = pupool.tile([128, S], FT, tag="pu")
                    for hh in range(HT // 2):
                        nc.tensor.matmul(
                            pg[:],
                            ge[:, 2 * hh : 2 * hh + 2, oi * 128 : (oi + 1) * 128],
                            xe[:, 2 * hh : 2 * hh + 2, :],
                            start=(hh == 0),
                            stop=(hh == HT // 2 - 1),
                            perf_mode=DR,
                        )
                    for hh in range(HT // 2):
                        nc.tensor.matmul(
                            pu[:],
                            ge[:, 2 * hh : 2 * hh + 2, 512 + oi * 128 : 512 + (oi + 1) * 128],
                            xe[:, 2 * hh : 2 * hh + 2, :],
                            start=(hh == 0),
                            stop=(hh == HT // 2 - 1),
                            perf_mode=DR,
                        )
                    sil = silpool.tile([128, S], FT, tag="sil")
                    nc.scalar.activation(sil[:], pg[:], silu_fn, scale=inv)
                    nc.vector.scalar_tensor_tensor(
                        mid[:, oi, :], pu[:], inv, sil[:],
                        mybir.AluOpType.mult, mybir.AluOpType.mult,
                    )

                y3 = ypool.tile([128, HT, S], BF, tag="yo")
                for h in range(HT):
                    py = pypool.tile([128, S], FT, tag="py")
                    for ii in range(IT // 2):
                        nc.tensor.matmul(
                            py[:],
                            de[:, 2 * ii : 2 * ii + 2, h * 128 : (h + 1) * 128],
                            mid[:, 2 * ii : 2 * ii + 2, :],
                            start=(ii == 0),
                            stop=(ii == IT // 2 - 1),
                            perf_mode=DR,
                        )
                    # split psum->bf16 copies across DVE and ACT
                    if h % 2 == 0:
                        nc.vector.tensor_copy(y3[:, h, :], py[:])
                    else:
                        nc.scalar.mul(y3[:, h, :], py[:], 1.0)
                        nc.gpsimd.dma_start(
                            yt_d[e, :, h - 1 : h + 1, :], y3[:, h - 1 : h + 1, :]
                        )

    nc.finalize()
    _BUILD_CACHE[key] = nc
    return nc


def _build_bf16(S):
    """bf16 fallback: identical math with K=128 matmuls (previous baseline)."""
    import concourse.bacc as bacc
    import concourse.bass as bass
    import concourse.mybir as mybir
    from concourse import tile

    key = (S, "bf16")
    if key in _BUILD_CACHE:
        return _BUILD_CACHE[key]

    FT = mybir.dt.float32
    BF = mybir.dt.bfloat16
    silu_fn = mybir.ActivationFunctionType.Silu

    chunks = _chunks(S)

    nc = bacc.Bacc(None)
    xt_d = nc.declare_dram_parameter("xt", [E_PER_CORE, 128, HT, S], BF, isOutput=False)
    gu_d = nc.declare_dram_parameter("guw", [E_PER_CORE, 128, HT, 1024], BF, isOutput=False)
    dn_d = nc.declare_dram_parameter("dnw", [E_PER_CORE, 128, IT, 1024], BF, isOutput=False)
    yt_d = nc.declare_dram_parameter("yt", [E_PER_CORE, 128, HT, S], BF, isOutput=True)

    with tile.TileContext(nc) as tc:
        with (
            tc.tile_pool(name="xpool", bufs=2) as xpool,
            tc.tile_pool(name="gupool", bufs=2) as gupool,
            tc.tile_pool(name="dnpool", bufs=2) as dnpool,
            tc.tile_pool(name="midpool", bufs=2) as midpool,
            tc.tile_pool(name="silpool", bufs=E_PER_CORE * IT * len(chunks)) as silpool,
            tc.tile_pool(name="ypool", bufs=2) as ypool,
            tc.tile_pool(name="pgpool", bufs=3, space="PSUM") as pgpool,
            tc.tile_pool(name="pupool", bufs=3, space="PSUM") as pupool,
            tc.tile_pool(name="pypool", bufs=2, space="PSUM") as pypool,
        ):
            for e in range(E_PER_CORE):
                xe = xpool.tile([128, HT, S], BF, tag="xt")
                ge = gupool.tile([128, HT, 1024], BF, tag="gu")
                de = dnpool.tile([128, IT, 1024], BF, tag="dn")
                for h in range(HT):
                    nc.sync.dma_start(ge[:, h, :], gu_d[e, :, h, :])
                    nc.scalar.dma_start(xe[:, h, :], xt_d[e, :, h, :])
                for i in range(IT):
                    nc.sync.dma_start(de[:, i, :], dn_d[e, :, i, :])

                mid = midpool.tile([128, IT, S], BF, tag="mid")
                for ci, (c0, cn) in enumerate(chunks):
                    for oi in range(IT):
                        pg = pgpool.tile([128, cn], FT, tag="pg")
                        pu = pupool.tile([128, cn], FT, tag="pu")
                        for h in range(HT):
                            nc.tensor.matmul(
                                pg[:],
                                ge[:, h, oi * 128 : (oi + 1) * 128],
                                xe[:, h, c0 : c0 + cn],
                                start=(h == 0),
                                stop=(h == HT - 1),
                            )
                        for h in range(HT):
                            nc.tensor.matmul(
                                pu[:],
                                ge[:, h, 512 + oi * 128 : 512 + (oi + 1) * 128],
                                xe[:, h, c0 : c0 + cn],
                                start=(h == 0),
                                stop=(h == HT - 1),
                            )
                        sil = silpool.tile([128, cn], FT, tag="sil")
                        nc.scalar.activation(sil[:], pg[:], silu_fn)
                        nc.vector.scalar_tensor_tensor(
                            mid[:, oi, c0 : c0 + cn], pu[:], 1.0, sil[:],
                            mybir.AluOpType.mult, mybir.AluOpType.mult,
                        )
                    y3 = ypool.tile([128, HT, S], BF, tag="yo") if ci == 0 else y3
                    for h in range(HT):
                        py = pypool.tile([128, cn], FT, tag="py")
                        for i in range(IT):
                            nc.tensor.matmul(
                                py[:],
                                de[:, i, h * 128 : (h + 1) * 128],
                                mid[:, i, c0 : c0 + cn],
                                start=(i == 0),
                                stop=(i == IT - 1),
                            )
                        if h % 2 == 0:
                            nc.vector.tensor_copy(y3[:, h, c0 : c0 + cn], py[:])
                        else:
                            nc.scalar.mul(y3[:, h, c0 : c0 + cn], py[:], 1.0)
                            if ci == len(chunks) - 1:
                                nc.gpsimd.dma_start(
                                    yt_d[e, :, h - 1 : h + 1, :], y3[:, h - 1 : h + 1, :]
                                )

    nc.finalize()
    _BUILD_CACHE[key] = nc
    return nc


def _install_trace_shims():
    """Make trace=True usable in this image: provide the NTFF hook module and
    neutralize the artifact upload (no bucket access needed for local use)."""
    import sys
    import types

    try:
        import antenv.axon_hooks  # noqa: F401
    except ImportError:
        hook = None
        try:
            from trn_agent_boot.trn_boot import _ntff_profile_via_ctypes

            hook = _ntff_profile_via_ctypes("/opt/axon/libaxon_pjrt.so")
        except Exception:
            hook = None
        mod = types.ModuleType("antenv.axon_hooks")
        mod._hook = hook
        mod.get_axon_ntff_profile_hook = lambda: mod._hook
        mod.set_axon_ntff_profile_hook = lambda h: setattr(mod, "_hook", h)
        sys.modules["antenv.axon_hooks"] = mod

    import concourse.bass_utils as bu

    orig_upload = bu.upload_artifacts

    def safe_upload(tmpdir):
        try:
            return orig_upload(tmpdir)
        except Exception:
            return tmpdir
    bu.upload_artifacts = safe_upload


def kernel(**inputs):
    import ml_dtypes
    from concourse.bass_utils import run_bass_kernel_spmd

    hidden = np.ascontiguousarray(np.asarray(inputs["hidden_states"], dtype=np.float32))
    idx = np.asarray(inputs["top_k_index"]).astype(np.int64)
    wts = np.asarray(inputs["top_k_weights"], dtype=np.float32)
    gup = np.asarray(inputs["gate_up_proj"], dtype=np.float32)
    dnp = np.asarray(inputs["down_proj"], dtype=np.float32)

    n_tok = hidden.shape[0]
    K = idx.shape[1]

    ve, vt, vw, vp, va, zero_w = _route(idx, wts, n_tok)
    cnts = np.bincount(ve, minlength=R)
    maxc = int(cnts.max())
    # N multiple of 64 elements keeps the PE moving-operand stream at full rate
    S = max(256, ((maxc + 63) // 64) * 64)

    if PREC == "fp8":
        io_np = ml_dtypes.float8_e4m3  # TRN FP8_EXP4: max +-240, IEEE-style
        wmul = WSCALE
    else:
        io_np = ml_dtypes.bfloat16
        wmul = 1.0

    estarts = np.cumsum(cnts) - cnts

    in_maps = []
    for c in range(N_CORES):
        xt = np.zeros((E_PER_CORE, 128, HT, S), dtype=io_np)
        for le in range(E_PER_CORE):
            ge = c * E_PER_CORE + le
            s0, cnt = estarts[ge], cnts[ge]
            if cnt == 0:
                continue
            toks = vt[s0 : s0 + cnt]
            # [cnt, H] -> [H, cnt] -> [HT, 128, cnt] -> [128, HT, cnt]
            xbuf = hidden[toks].T.reshape(HT, 128, cnt).transpose(1, 0, 2)
            xt[le, :, :, :cnt] = xbuf.astype(io_np)
        guw = (
            (gup[c * E_PER_CORE : (c + 1) * E_PER_CORE] * wmul)
            .transpose(0, 2, 1)  # [4, H, 2I]
            .reshape(E_PER_CORE, HT, 128, 1024)
            .transpose(0, 2, 1, 3)  # [4, 128, HT, 1024]
            .astype(io_np)
        )
        dnw = (
            (dnp[c * E_PER_CORE : (c + 1) * E_PER_CORE] * wmul)
            .transpose(0, 2, 1)  # [4, I, H]
            .reshape(E_PER_CORE, IT, 128, 1024)
            .transpose(0, 2, 1, 3)  # [4, 128, IT, 1024]
            .astype(io_np)
        )
        in_maps.append({"xt": np.ascontiguousarray(xt),
                        "guw": np.ascontiguousarray(guw),
                        "dnw": np.ascontiguousarray(dnw)})

    nc = _build_fp8(S) if PREC == "fp8" else _build_bf16(S)

    trace = bool(int(os.environ.get("KERNEL_TRACE", "0")))
    if trace:
        _install_trace_shims()
    res = run_bass_kernel_spmd(nc, in_maps, list(range(N_CORES)), trace=trace)
    LAST_RUN["exec_time_ns"] = res.exec_time_ns
    LAST_RUN["mean_exec_time_ns"] = res.mean_exec_time_ns
    LAST_RUN["instructions_and_trace"] = res.instructions_and_trace
    LAST_RUN["profile_json"] = res.profile_json

    # ---- combine on host ----
    yscale = 1.0 / WSCALE if PREC == "fp8" else 1.0
    out = hidden * zero_w[:, None].astype(np.float32)
    acc = np.zeros((n_tok * K, H), dtype=np.float32)
    for c in range(N_CORES):
        yt = np.asarray(res.results[c]["yt"]).astype(np.float32)  # [4, 128, HT, S]
        for le in range(E_PER_CORE):
            ge = c * E_PER_CORE + le
            s0, cnt = estarts[ge], cnts[ge]
            if cnt == 0:
                continue
            # [128, HT, S] -> [HT, 128, S] -> [H, S]
            y = yt[le].transpose(1, 0, 2).reshape(H, S)[:, :cnt].T
            acc[va[s0 : s0 + cnt]] = y * (vw[s0 : s0 + cnt, None] * yscale)
    out += acc.reshape(n_tok, K, H).sum(axis=1)
    return out
